# revision 1
# baseline (speedup 1.0000x reference)
"""Single-head causal attention on 8 Trainium2 NeuronCores (Bass/Tile).

Problem: x[4, 2048, 1024], Wq/Wk/Wv[1024, 1024] fp32 ->
         softmax(causal(q k^T / sqrt(1024))) v,  q/k/v = x @ W.

Sharding (uniform SPMD, one NEFF for all 8 cores):
  - 2 cores per batch. Each core computes the full K/V projection for its
    batch (recompute instead of collectives) plus attention for a balanced,
    interleaved quarter of all query rows (1024 rows/core).
  - Query 256-row groups 0..7 of the sequence are split between the batch
    pair as {7,4,3,0} (even core) / {6,5,2,1} (odd core): both sets need
    exactly the same number of causal key tiles per slot position, so the
    compiled kernel is identical across cores; only input data differs.
  - Per-slot key-tile bounds are the uniform elementwise max [16,12,8,4];
    the last 4 key tiles of each slot get a host-computed 0/1 mask input
    (causal + zero padding), everything earlier is causally full.

Kernel structure per core:
  Phase A: transpose x on-chip (PE transpose) -> xT; project kT = Wk^T x^T
           ([e,s] layout) and v = x Wv ([s,e]); spill both to DRAM scratch.
  Phase B: same for the core's own 1024 query rows -> qT resident in SBUF.
  Phase C: flash-style attention, key-block (512) outer loop:
           scoresT[k,q] = kT^T qT (PSUM, fp32 accum), exp on ACT engine
           (scale=1/32 folded in), causal mask via 0/1 multiply, then
           O += w^T V and row-sums l += w^T 1 on the PE; final out = O/l.

All matmuls run as float32r (TF32-class fast fp32 path, 1 cyc/row at
moving dim >= 256) end-to-end; PSUM accumulation is fp32.
"""

import sys

import numpy as np

for _p in ("/opt/trn_rl_repo", "/root/.axon_site/_ro/trn_rl_repo"):
    if _p not in sys.path:
        sys.path.append(_p)

B, S, D = 4, 2048, 1024
P = 128
QL = 1024  # query rows per core
GW = 256  # query group width
NSLOT = 4  # query groups per core
BOUNDS = (16, 12, 8, 4)  # uniform per-slot key-tile bounds
GROUPS = {0: (7, 4, 3, 0), 1: (6, 5, 2, 1)}  # 256-row q-groups per parity
NMASK = 4  # masked key tiles per slot (the last 4)
SCALE = 1.0 / np.sqrt(np.float32(D))

_cached = {}


def _build_bass():
    import concourse.bacc as bacc
    import concourse.mybir as mybir
    import concourse.tile as tile
    from concourse.masks import make_identity
    from contextlib import ExitStack

    f32 = mybir.dt.float32
    f32r = mybir.dt.float32r

    nc = bacc.Bacc("TRN2")
    x_d = nc.declare_dram_parameter("x", [S, D], f32r, isOutput=False)
    xq_d = nc.declare_dram_parameter("xq", [QL, D], f32r, isOutput=False)
    wq_d = nc.declare_dram_parameter("Wq", [D, D], f32r, isOutput=False)
    wk_d = nc.declare_dram_parameter("Wk", [D, D], f32r, isOutput=False)
    wv_d = nc.declare_dram_parameter("Wv", [D, D], f32r, isOutput=False)
    masks_d = nc.declare_dram_parameter(
        "masks", [NSLOT, NMASK, P, GW], f32r, isOutput=False
    )
    out_d = nc.declare_dram_parameter("out", [QL, D], f32, isOutput=True)

    kT_dram = nc.dram_tensor("kT_scratch", [D, S], f32r)  # [e, s]
    v_dram = nc.dram_tensor("v_scratch", [S, D], f32r)  # [s, e]

    DT = D // P  # 8 d-tiles
    ET = D // P  # 8 e-tiles
    SB = 512  # s-block for projections
    KB = 512  # key block in attention
    NKB = S // KB  # 4 key blocks

    with tile.TileContext(nc, pool_alloc_mode="queue") as tc, ExitStack() as top:
        cpool = top.enter_context(tc.tile_pool(name="const", bufs=1))
        ident_f = cpool.tile([P, P], f32)
        make_identity(nc, ident_f)
        ident = cpool.tile([P, P], f32r)
        nc.vector.tensor_copy(ident, ident_f)
        ones_f = cpool.tile([P, 2], f32)
        nc.gpsimd.memset(ones_f, 1.0)
        ones = cpool.tile([P, 2], f32r)
        nc.vector.tensor_copy(ones, ones_f)

        # qT stays resident from phase B through phase C: [e, q] layout.
        qT_pool = top.enter_context(tc.tile_pool(name="qT", bufs=1))
        qT = [qT_pool.tile([P, QL], f32r, name=f"qT{e}") for e in range(ET)]

        def load_xn(xn_pool, src_dram, row0):
            xns = []
            for st in range(4):
                xn = xn_pool.tile([P, D], f32r, tag="xn", name=f"xn{st}")
                nc.sync.dma_start(xn, src_dram[row0 + st * P : row0 + (st + 1) * P, :])
                xns.append(xn)
            return xns

        def do_transposes(tpsum, xns, xT_tiles):
            for dt in range(DT):
                for st in range(4):
                    tp = tpsum.tile([P, P], f32r, tag="tp", name="tp")
                    nc.tensor.transpose(tp, xns[st][:, dt * P : (dt + 1) * P], ident)
                    nc.vector.tensor_copy(xT_tiles[dt][:, st * P : (st + 1) * P], tp)

        # ---------------- Phase A: kT / v projection of full x ----------------
        with ExitStack() as pa:
            xn_pool = pa.enter_context(tc.tile_pool(name="xnA", bufs=8))
            xT_pool = pa.enter_context(tc.tile_pool(name="xTA", bufs=2))
            tpsum = pa.enter_context(tc.tile_pool(name="tpsA", bufs=3, space="PSUM"))
            ppool = pa.enter_context(tc.tile_pool(name="ppsA", bufs=5, space="PSUM"))
            stage = pa.enter_context(tc.tile_pool(name="stgA", bufs=6))

            xns_next = load_xn(xn_pool, x_d, 0)

            wkv_pool = pa.enter_context(tc.tile_pool(name="wkv", bufs=1))
            wk_sb = [wkv_pool.tile([P, D], f32r, name=f"wk{d}") for d in range(DT)]
            wv_sb = [wkv_pool.tile([P, D], f32r, name=f"wv{d}") for d in range(DT)]
            for d in range(DT):
                nc.sync.dma_start(wk_sb[d], wk_d[d * P : (d + 1) * P, :])
            for d in range(DT):
                nc.sync.dma_start(wv_sb[d], wv_d[d * P : (d + 1) * P, :])

            for sb in range(S // SB):
                xns = xns_next
                if sb + 1 < S // SB:
                    xns_next = load_xn(xn_pool, x_d, (sb + 1) * SB)
                xT = [
                    xT_pool.tile([P, SB], f32r, tag=f"xT{dt}", name=f"xT{dt}")
                    for dt in range(DT)
                ]
                do_transposes(tpsum, xns, xT)

                # kT[e, s] = sum_d Wk[d,e]^T x^T[d,s]
                for et in range(ET):
                    pk = ppool.tile([P, SB], f32, tag="pp", name="pk")
                    for dt in range(DT):
                        nc.tensor.matmul(
                            pk,
                            lhsT=wk_sb[dt][:, et * P : (et + 1) * P],
                            rhs=xT[dt],
                            start=(dt == 0),
                            stop=(dt == DT - 1),
                        )
                    ks = stage.tile([P, SB], f32r, tag="stg", name="ks")
                    nc.scalar.copy(ks, pk)
                    nc.sync.dma_start(
                        kT_dram[et * P : (et + 1) * P, sb * SB : (sb + 1) * SB], ks
                    )
                # v[s, e] = sum_d (x^T[d,s])^T Wv[d,e]
                for st in range(4):
                    for eh in range(2):
                        pv = ppool.tile([P, SB], f32, tag="pp", name="pv")
                        for dt in range(DT):
                            nc.tensor.matmul(
                                pv,
                                lhsT=xT[dt][:, st * P : (st + 1) * P],
                                rhs=wv_sb[dt][:, eh * 512 : (eh + 1) * 512],
                                start=(dt == 0),
                                stop=(dt == DT - 1),
                            )
                        vs = stage.tile([P, SB], f32r, tag="stg", name="vs")
                        nc.scalar.copy(vs, pv)
                        nc.sync.dma_start(
                            v_dram[
                                sb * SB + st * P : sb * SB + (st + 1) * P,
                                eh * 512 : (eh + 1) * 512,
                            ],
                            vs,
                        )

        # ---------------- Phase B: qT projection of own query rows ----------------
        with ExitStack() as pb:
            wq_pool = pb.enter_context(tc.tile_pool(name="wq", bufs=1))
            wq_sb = [wq_pool.tile([P, D], f32r, name=f"wq{d}") for d in range(DT)]
            for d in range(DT):
                nc.sync.dma_start(wq_sb[d], wq_d[d * P : (d + 1) * P, :])

            xn_pool = pb.enter_context(tc.tile_pool(name="xnB", bufs=8))
            xT_pool = pb.enter_context(tc.tile_pool(name="xTB", bufs=2))
            tpsum = pb.enter_context(tc.tile_pool(name="tpsB", bufs=4, space="PSUM"))
            ppool = pb.enter_context(tc.tile_pool(name="ppsB", bufs=4, space="PSUM"))

            xns_next = load_xn(xn_pool, xq_d, 0)
            for sb in range(QL // SB):
                xns = xns_next
                if sb + 1 < QL // SB:
                    xns_next = load_xn(xn_pool, xq_d, (sb + 1) * SB)
                xT = [
                    xT_pool.tile([P, SB], f32r, tag=f"xTq{dt}", name=f"xTq{dt}")
                    for dt in range(DT)
                ]
                do_transposes(tpsum, xns, xT)
                for et in range(ET):
                    pq = ppool.tile([P, SB], f32, tag="ppq", name="pq")
                    for dt in range(DT):
                        nc.tensor.matmul(
                            pq,
                            lhsT=wq_sb[dt][:, et * P : (et + 1) * P],
                            rhs=xT[dt],
                            start=(dt == 0),
                            stop=(dt == DT - 1),
                        )
                    nc.scalar.copy(qT[et][:, sb * SB : (sb + 1) * SB], pq)

        # ---------------- Phase C: attention ----------------
        with ExitStack() as pc:
            mpool = pc.enter_context(tc.tile_pool(name="masks", bufs=1))
            masks_sb = mpool.tile([P, NSLOT * NMASK * GW], f32r)
            for j in range(NSLOT):
                for m in range(NMASK):
                    col = (j * NMASK + m) * GW
                    nc.sync.dma_start(masks_sb[:, col : col + GW], masks_d[j, m])

            acc_pool = pc.enter_context(tc.tile_pool(name="acc", bufs=1))
            # O accumulators: per (slot, q-half) a [128, 1024] fp32 tile.
            O_sb = [
                [acc_pool.tile([P, D], f32, name=f"O{j}_{h}") for h in range(2)]
                for j in range(NSLOT)
            ]
            l_sb = acc_pool.tile([P, 2 * NSLOT], f32)  # row sums, col = 2*slot+half

            kv_pool = pc.enter_context(tc.tile_pool(name="kv", bufs=2))
            # deeper exp->AV pipeline
            w_pool = pc.enter_context(tc.tile_pool(name="wT", bufs=8))
            fin_pool = pc.enter_context(tc.tile_pool(name="fin", bufs=4))
            rec_pool = pc.enter_context(tc.tile_pool(name="recp", bufs=4))
            spsum = pc.enter_context(tc.tile_pool(name="sps", bufs=3, space="PSUM"))
            opsum = pc.enter_context(tc.tile_pool(name="ops", bufs=4, space="PSUM"))
            lpsum = pc.enter_context(tc.tile_pool(name="lps", bufs=1, space="PSUM"))

            for kb in range(NKB):
                kT_sb = [
                    kv_pool.tile([P, KB], f32r, tag=f"kT{e}", name=f"kTs{e}")
                    for e in range(ET)
                ]
                for e in range(ET):
                    nc.sync.dma_start(
                        kT_sb[e], kT_dram[e * P : (e + 1) * P, kb * KB : (kb + 1) * KB]
                    )
                v_sb = [
                    kv_pool.tile([P, D], f32r, tag=f"v{t}", name=f"vs{t}")
                    for t in range(4)
                ]
                for t in range(4):
                    nc.sync.dma_start(
                        v_sb[t],
                        v_dram[kb * KB + t * P : kb * KB + (t + 1) * P, :],
                    )

                for j in range(NSLOT):
                    if kb * 4 >= BOUNDS[j]:
                        continue
                    qcol = j * GW
                    wts = []
                    for kt in range(4):
                        K = kb * 4 + kt  # global key tile
                        sp = spsum.tile([P, GW], f32, tag="sp", name="sp")
                        for e in range(ET):
                            nc.tensor.matmul(
                                sp,
                                lhsT=kT_sb[e][:, kt * P : (kt + 1) * P],
                                rhs=qT[e][:, qcol : qcol + GW],
                                start=(e == 0),
                                stop=(e == ET - 1),
                            )
                        wt = w_pool.tile([P, GW], f32r, tag="wt", name="wt")
                        # w = exp(scores / sqrt(d_k)); scale folded into ACT.
                        nc.scalar.activation(
                            wt, sp, mybir.ActivationFunctionType.Exp, scale=float(SCALE)
                        )
                        m = K - (BOUNDS[j] - NMASK)
                        if m >= 0:
                            mcol = (j * NMASK + m) * GW
                            nc.vector.tensor_mul(
                                wt, wt, masks_sb[:, mcol : mcol + GW]
                            )
                        wts.append(wt)

                    lp = lpsum.tile([P, 4], f32, tag="lp", name="lp")
                    for h in range(2):
                        for eh in range(2):
                            op = opsum.tile([P, 512], f32, tag="op", name="op")
                            for kt in range(4):
                                nc.tensor.matmul(
                                    op,
                                    lhsT=wts[kt][:, h * P : (h + 1) * P],
                                    rhs=v_sb[kt][:, eh * 512 : (eh + 1) * 512],
                                    start=(kt == 0),
                                    stop=(kt == 3),
                                )
                            dst = O_sb[j][h][:, eh * 512 : (eh + 1) * 512]
                            if kb == 0:
                                nc.vector.tensor_copy(dst, op)
                            else:
                                nc.vector.tensor_add(dst, dst, op)
                        for kt in range(4):
                            nc.tensor.matmul(
                                lp[:, 2 * h : 2 * h + 2],
                                lhsT=wts[kt][:, h * P : (h + 1) * P],
                                rhs=ones,
                                start=(kt == 0),
                                stop=(kt == 3),
                            )
                    for h in range(2):
                        lcol = l_sb[:, 2 * j + h : 2 * j + h + 1]
                        if kb == 0:
                            nc.vector.tensor_copy(lcol, lp[:, 2 * h : 2 * h + 1])
                        else:
                            nc.vector.tensor_add(lcol, lcol, lp[:, 2 * h : 2 * h + 1])

                    if kb == BOUNDS[j] // 4 - 1:
                        # last key block for this slot: normalize + store now
                        recip = rec_pool.tile([P, 2], f32, tag="rc", name="recip")
                        nc.vector.reciprocal(recip, l_sb[:, 2 * j : 2 * j + 2])
                        for h in range(2):
                            o = fin_pool.tile([P, D], f32, tag="fo", name="fo")
                            nc.scalar.activation(
                                o,
                                O_sb[j][h],
                                mybir.ActivationFunctionType.Copy,
                                scale=recip[:, h : h + 1],
                            )
                            row = j * GW + h * P
                            nc.sync.dma_start(out_d[row : row + P, :], o)

    nc.compile()
    return nc


def _host_inputs(x, Wq, Wk, Wv):
    in_maps = []
    for c in range(8):
        b, par = c // 2, c % 2
        groups = GROUPS[par]
        rows = np.concatenate(
            [np.arange(GW * g, GW * g + GW) for g in groups]
        )
        xq = np.ascontiguousarray(x[b][rows])
        masks = np.zeros((NSLOT, NMASK, P, GW), np.float32)
        for j, g in enumerate(groups):
            bj = BOUNDS[j]
            for m, kt in enumerate(range(bj - NMASK, bj)):
                kg = P * kt + np.arange(P)[:, None]
                qg = GW * g + np.arange(GW)[None, :]
                masks[j, m] = (kg <= qg).astype(np.float32)
        in_maps.append(
            {
                "x": np.ascontiguousarray(x[b]),
                "xq": xq,
                "Wq": Wq,
                "Wk": Wk,
                "Wv": Wv,
                "masks": masks,
            }
        )
    return in_maps


def kernel(x, Wq, Wk, Wv):
    from concourse.bass_utils import run_bass_kernel_spmd

    x = np.asarray(x, dtype=np.float32)
    Wq = np.ascontiguousarray(np.asarray(Wq, dtype=np.float32))
    Wk = np.ascontiguousarray(np.asarray(Wk, dtype=np.float32))
    Wv = np.ascontiguousarray(np.asarray(Wv, dtype=np.float32))

    if "nc" not in _cached:
        _cached["nc"] = _build_bass()
    nc = _cached["nc"]

    in_maps = _host_inputs(x, Wq, Wk, Wv)
    res = run_bass_kernel_spmd(nc, in_maps, core_ids=list(range(8)))
    _cached["last_result"] = res

    out = np.zeros((B, S, D), np.float32)
    for c in range(8):
        b, par = c // 2, c % 2
        oc = res.results[c]["out"]
        for j, g in enumerate(GROUPS[par]):
            out[b, GW * g : GW * g + GW] = oc[GW * j : GW * j + GW]
    return out



# revision 2
# speedup vs baseline: 1.1772x; 1.1772x over previous
"""Single-head causal attention on 8 Trainium2 NeuronCores (Bass/Tile), v3.

v2 -> v3: projections run in bf16 (x, xq, Wq, Wk, Wv shipped as bf16; PSUM
accumulation stays fp32; attention matmuls stay float32r), which halves the
head DMA that starved the PE in v2; DMA issue order and emission order are
interleaved so the q-phase and the first key half-block start as their data
lands; the final normalize/store is split in halves to shorten the tail.

See kernel_v2.py docstring for the sharding + fused-loop design.
"""

import sys

import numpy as np

for _p in ("/opt/trn_rl_repo", "/root/.axon_site/_ro/trn_rl_repo"):
    if _p not in sys.path:
        sys.path.append(_p)

B, S, D = 4, 2048, 1024
P = 128
QL = 1024  # query rows per core
NSLOT = 8  # query slots (128 rows each) per core
NHB = 8  # key half-blocks of 256
BOUNDS = tuple(2 * (8 - j) for j in range(NSLOT))  # (16,14,...,2) key tiles
GROUPS = {
    0: (15, 12, 11, 8, 7, 4, 3, 0),  # slot -> global q-tile, even cores
    1: (14, 13, 10, 9, 6, 5, 2, 1),  # odd cores
}
SCALE = 1.0 / np.sqrt(np.float32(D))

_cached = {}


def _build_bass():
    import concourse.bacc as bacc
    import concourse.mybir as mybir
    import concourse.tile as tile
    from contextlib import ExitStack

    f32 = mybir.dt.float32
    f32r = mybir.dt.float32r
    bf16 = mybir.dt.bfloat16

    nc = bacc.Bacc("TRN2")
    # Host-side layouts (partition-major, every DMA a contiguous [128,N]):
    #   xT:    [128, hb*2048 + dt*256 + c]  (x^T tiles [dt, hb] of [128,256])
    #   xqT:   [128, sb*4096 + dt*512 + c]  (own q rows^T, [dt, sb] blocks)
    #   W*:    [128, dt*1024 + e]           (weight d-tiles side by side)
    #   masks: [128, (2*hb+kt)*128 + c]     (finishing slot's 2 causal tiles)
    xt_d = nc.declare_dram_parameter("xT", [P, NHB * 2048], bf16, isOutput=False)
    xq_d = nc.declare_dram_parameter("xqT", [P, 2 * 4096], bf16, isOutput=False)
    wq_d = nc.declare_dram_parameter("Wq", [P, 8 * D], bf16, isOutput=False)
    wk_d = nc.declare_dram_parameter("Wk", [P, 8 * D], bf16, isOutput=False)
    wv_d = nc.declare_dram_parameter("Wv", [P, 8 * D], bf16, isOutput=False)
    masks_d = nc.declare_dram_parameter("masks", [P, 16 * P], f32r, isOutput=False)
    out_d = nc.declare_dram_parameter("out", [QL, D], f32, isOutput=True)

    DT = D // P  # 8 d-tiles
    ET = D // P  # 8 e-tiles

    with tile.TileContext(nc, pool_alloc_mode="queue") as tc, ExitStack() as top:
        cpool = top.enter_context(tc.tile_pool(name="const", bufs=1))
        ones_f = cpool.tile([P, 2], f32)
        nc.gpsimd.memset(ones_f, 1.0)
        ones = cpool.tile([P, 2], f32r)
        nc.vector.tensor_copy(ones, ones_f)

        # Persistent SBUF residents.
        qT_pool = top.enter_context(tc.tile_pool(name="qT", bufs=1))
        qT = [qT_pool.tile([P, QL], f32r, name=f"qT{e}") for e in range(ET)]
        wkv_pool = top.enter_context(tc.tile_pool(name="wkv", bufs=1))
        wk_sb = wkv_pool.tile([P, 8 * D], bf16, name="wk")
        wv_sb = wkv_pool.tile([P, 8 * D], bf16, name="wv")
        mpool = top.enter_context(tc.tile_pool(name="masks", bufs=1))
        masks_sb = mpool.tile([P, 16 * P], f32r)
        acc_pool = top.enter_context(tc.tile_pool(name="acc", bufs=1))
        O_sb = [acc_pool.tile([P, D], f32, name=f"O{j}") for j in range(NSLOT)]
        l_sb = acc_pool.tile([P, NSLOT], f32)
        xTp = top.enter_context(tc.tile_pool(name="xT", bufs=1))
        xT_tiles = [xTp.tile([P, 2048], bf16, name=f"xT{h}") for h in range(NHB)]

        # ---------------- Phase Q: qT projection ----------------
        with ExitStack() as pq_scope:
            wq_pool = pq_scope.enter_context(tc.tile_pool(name="wq", bufs=1))
            wq_sb = wq_pool.tile([P, 8 * D], bf16)
            xq_pool = pq_scope.enter_context(tc.tile_pool(name="xq", bufs=1))
            xq_sb = xq_pool.tile([P, 2 * 4096], bf16)

            # DMA issue order = data-need order; 1MB chunks so compute can
            # start as soon as the first chunk lands.
            nc.sync.dma_start(xq_sb[:, 0:1024], xq_d[:, 0:1024])
            nc.sync.dma_start(wq_sb[:, 0:1024], wq_d[:, 0:1024])
            nc.sync.dma_start(xq_sb[:, 1024:2048], xq_d[:, 1024:2048])
            nc.sync.dma_start(wq_sb[:, 1024:2048], wq_d[:, 1024:2048])
            nc.sync.dma_start(xq_sb[:, 2048:4096], xq_d[:, 2048:4096])
            nc.sync.dma_start(wq_sb[:, 2048:4096], wq_d[:, 2048:4096])
            nc.sync.dma_start(xq_sb[:, 4096:8192], xq_d[:, 4096:8192])
            nc.sync.dma_start(wq_sb[:, 4096:8192], wq_d[:, 4096:8192])
            nc.sync.dma_start(wk_sb[:, 0:4096], wk_d[:, 0:4096])
            nc.sync.dma_start(wk_sb[:, 4096:8192], wk_d[:, 4096:8192])
            nc.sync.dma_start(xT_tiles[0], xt_d[:, 0:2048])
            nc.sync.dma_start(wv_sb[:, 0:4096], wv_d[:, 0:4096])
            nc.sync.dma_start(wv_sb[:, 4096:8192], wv_d[:, 4096:8192])
            nc.sync.dma_start(masks_sb, masks_d[:, :])
            for _hb in range(1, NHB):
                _xt = xT_tiles[_hb]
                nc.sync.dma_start(_xt, xt_d[:, _hb * 2048 : (_hb + 1) * 2048])

            ppq = pq_scope.enter_context(tc.tile_pool(name="ppq", bufs=6, space="PSUM"))
            for sb in range(2):
                for et in range(ET):
                    pq = ppq.tile([P, 512], f32, tag="ppq", name="pq")
                    for dt in range(DT):
                        nc.tensor.matmul(
                            pq,
                            lhsT=wq_sb[:, dt * D + et * P : dt * D + (et + 1) * P],
                            rhs=xq_sb[:, sb * 4096 + dt * 512 : sb * 4096 + (dt + 1) * 512],
                            start=(dt == 0),
                            stop=(dt == DT - 1),
                        )
                    nc.scalar.copy(qT[et][:, sb * 512 : (sb + 1) * 512], pq)

        # ---------------- Main loop: fused K/V projection + attention ----------------
        with ExitStack() as mn:
            kv_pool = mn.enter_context(tc.tile_pool(name="kv", bufs=2))
            wt_pool = mn.enter_context(tc.tile_pool(name="wt", bufs=4))
            fin_pool = mn.enter_context(tc.tile_pool(name="fin", bufs=2))
            rec_pool = mn.enter_context(tc.tile_pool(name="rec", bufs=2))
            ps_pool = mn.enter_context(tc.tile_pool(name="ps", bufs=7, space="PSUM"))
            lp_pool = mn.enter_context(tc.tile_pool(name="lp", bufs=1, space="PSUM"))

            for hb in range(NHB):
                n = NSLOT - hb  # active slot prefix length
                xT_hb = xT_tiles[hb]

                # kT[e, s] for this half-block: 8 tiles [128, 256]
                kTs = []
                for et in range(ET):
                    pk = ps_pool.tile([P, 512], f32, tag="ps", name="pk")
                    for dt in range(DT):
                        nc.tensor.matmul(
                            pk[:, 0:256],
                            lhsT=wk_sb[:, dt * D + et * P : dt * D + (et + 1) * P],
                            rhs=xT_hb[:, dt * 256 : (dt + 1) * 256],
                            start=(dt == 0),
                            stop=(dt == DT - 1),
                        )
                    kt_sb = kv_pool.tile([P, 256], f32r, tag=f"kT{et}", name=f"kT{et}")
                    nc.scalar.copy(kt_sb, pk[:, 0:256])
                    kTs.append(kt_sb)

                # v[s, e] for this half-block: 2 tiles [128, 1024]
                vs = []
                for st in range(2):
                    v_sb = kv_pool.tile([P, D], f32r, tag=f"v{st}", name=f"v{st}")
                    for eh in range(2):
                        pv = ps_pool.tile([P, 512], f32, tag="ps", name="pv")
                        for dt in range(DT):
                            nc.tensor.matmul(
                                pv,
                                lhsT=xT_hb[:, dt * 256 + st * P : dt * 256 + (st + 1) * P],
                                rhs=wv_sb[:, dt * D + eh * 512 : dt * D + (eh + 1) * 512],
                                start=(dt == 0),
                                stop=(dt == DT - 1),
                            )
                        nc.scalar.copy(v_sb[:, eh * 512 : (eh + 1) * 512], pv)
                    vs.append(v_sb)

                # scoresT + exp for the two key tiles of this half-block.
                width = P * n
                wpad = max(width, 256)  # keep moving dim >= 256 for f32r speed
                wts = []
                for kt in range(2):
                    wt = wt_pool.tile([P, D], f32r, tag="wt", name="wt")
                    for c0 in range(0, wpad, 512):
                        cw = min(512, wpad - c0)
                        sp = ps_pool.tile([P, 512], f32, tag="ps", name="sp")
                        for et in range(ET):
                            nc.tensor.matmul(
                                sp[:, 0:cw],
                                lhsT=kTs[et][:, kt * P : (kt + 1) * P],
                                rhs=qT[et][:, c0 : c0 + cw],
                                start=(et == 0),
                                stop=(et == ET - 1),
                            )
                        nc.scalar.activation(
                            wt[:, c0 : c0 + cw],
                            sp[:, 0:cw],
                            mybir.ActivationFunctionType.Exp,
                            scale=float(SCALE),
                        )
                    wts.append(wt)
                # Causal mask: only the finishing slot (j = n-1) is partial.
                for kt in range(2):
                    nc.vector.tensor_mul(
                        wts[kt][:, (n - 1) * P : n * P],
                        wts[kt][:, (n - 1) * P : n * P],
                        masks_sb[:, (2 * hb + kt) * P : (2 * hb + kt + 1) * P],
                    )

                # O_j += w^T V (PSUM-accumulated over the 2 key tiles), l_j += w^T 1.
                for j in range(n):
                    last = j == n - 1
                    if last:
                        lp = lp_pool.tile([P, 2], f32, tag="lp", name="lp")
                        for kt in range(2):
                            nc.tensor.matmul(
                                lp,
                                lhsT=wts[kt][:, j * P : (j + 1) * P],
                                rhs=ones,
                                start=(kt == 0),
                                stop=(kt == 1),
                            )
                        lcol = l_sb[:, j : j + 1]
                        if hb == 0:
                            nc.vector.tensor_copy(lcol, lp[:, 0:1])
                        else:
                            nc.vector.tensor_add(lcol, lcol, lp[:, 0:1])
                        rec = rec_pool.tile([P, 1], f32, tag="rc", name="rec")
                        nc.vector.reciprocal(rec, l_sb[:, j : j + 1])
                        fo = fin_pool.tile([P, D], f32, tag="fo", name="fo")
                    for eh in range(2):
                        op = ps_pool.tile([P, 512], f32, tag="ps", name="op")
                        for kt in range(2):
                            nc.tensor.matmul(
                                op,
                                lhsT=wts[kt][:, j * P : (j + 1) * P],
                                rhs=vs[kt][:, eh * 512 : (eh + 1) * 512],
                                start=(kt == 0),
                                stop=(kt == 1),
                            )
                        dst = O_sb[j][:, eh * 512 : (eh + 1) * 512]
                        if hb == 0:
                            nc.vector.tensor_copy(dst, op)
                        else:
                            nc.vector.tensor_add(dst, dst, op)
                        if last:
                            # Slot finished: normalize + store this half now.
                            cwf = 512
                            for f0 in range(eh * 512, (eh + 1) * 512, cwf):
                                nc.scalar.activation(
                                    fo[:, f0 : f0 + cwf],
                                    O_sb[j][:, f0 : f0 + cwf],
                                    mybir.ActivationFunctionType.Copy,
                                    scale=rec[:, 0:1],
                                )
                                nc.sync.dma_start(
                                    out_d[j * P : (j + 1) * P, f0 : f0 + cwf],
                                    fo[:, f0 : f0 + cwf],
                                )
                    if not last:
                        lp = lp_pool.tile([P, 2], f32, tag="lp", name="lp")
                        for kt in range(2):
                            nc.tensor.matmul(
                                lp,
                                lhsT=wts[kt][:, j * P : (j + 1) * P],
                                rhs=ones,
                                start=(kt == 0),
                                stop=(kt == 1),
                            )
                        lcol = l_sb[:, j : j + 1]
                        if hb == 0:
                            nc.vector.tensor_copy(lcol, lp[:, 0:1])
                        else:
                            nc.vector.tensor_add(lcol, lcol, lp[:, 0:1])

    nc.compile()
    return nc


def _host_inputs(x, Wq, Wk, Wv):
    import ml_dtypes

    bf16 = ml_dtypes.bfloat16

    def wlayout(w):
        # [1024, 1024] -> [128, dt*1024 + e]
        return np.ascontiguousarray(
            w.reshape(8, P, D).transpose(1, 0, 2).reshape(P, 8 * D).astype(bf16)
        )

    wq_h, wk_h, wv_h = wlayout(Wq), wlayout(Wk), wlayout(Wv)

    in_maps = []
    for c in range(8):
        b, par = c // 2, c % 2
        groups = GROUPS[par]
        xb = x[b]  # [S, D]
        xT = xb.T  # [D, S]
        # xT: [128, hb*2048 + dt*256 + c]
        xt_h = np.ascontiguousarray(
            xT.reshape(8, P, 8, 256).transpose(1, 2, 0, 3).reshape(P, NHB * 2048).astype(bf16)
        )
        # own query rows (slot-ordered), transposed: [128, sb*4096 + dt*512 + c]
        rows = np.concatenate([np.arange(P * g, P * g + P) for g in groups])
        xqT = np.ascontiguousarray(xb[rows].T)  # [D, QL]
        xq_h = np.ascontiguousarray(
            xqT.reshape(8, P, 2, 512).transpose(1, 2, 0, 3).reshape(P, 8192).astype(bf16)
        )
        # masks: [128, (2*hb+kt)*128 + c]; finishing slot j=7-hb, K=2hb+kt
        masks = np.zeros((P, 16 * P), np.float32)
        for hb in range(NHB):
            j = 7 - hb
            g = groups[j]
            for kt in range(2):
                K = 2 * hb + kt
                kg = P * K + np.arange(P)[:, None]
                qg = P * g + np.arange(P)[None, :]
                masks[:, (2 * hb + kt) * P : (2 * hb + kt + 1) * P] = (
                    kg <= qg
                ).astype(np.float32)
        in_maps.append(
            {
                "xT": xt_h,
                "xqT": xq_h,
                "Wq": wq_h,
                "Wk": wk_h,
                "Wv": wv_h,
                "masks": masks,
            }
        )
    return in_maps


def kernel(x, Wq, Wk, Wv):
    from concourse.bass_utils import run_bass_kernel_spmd

    x = np.asarray(x, dtype=np.float32)
    Wq = np.ascontiguousarray(np.asarray(Wq, dtype=np.float32))
    Wk = np.ascontiguousarray(np.asarray(Wk, dtype=np.float32))
    Wv = np.ascontiguousarray(np.asarray(Wv, dtype=np.float32))

    if "nc" not in _cached:
        _cached["nc"] = _build_bass()
    nc = _cached["nc"]

    in_maps = _host_inputs(x, Wq, Wk, Wv)
    res = run_bass_kernel_spmd(nc, in_maps, core_ids=list(range(8)))
    _cached["last_result"] = res

    out = np.zeros((B, S, D), np.float32)
    for c in range(8):
        b, par = c // 2, c % 2
        oc = res.results[c]["out"]
        for j, g in enumerate(GROUPS[par]):
            out[b, P * g : P * g + P] = oc[P * j : P * (j + 1)]
    return out
